# revision 23
# baseline (speedup 1.0000x reference)
"""Trainium2 Bass kernel for Clique2NodeConvBasic (GNN message passing).

Computes, for the fixed problem size N=100000 nodes, C=50000 cliques,
E=1600000 edges, D=128:

    gathered = x_clique[clique_idx]            # [E, 128]
    summed   = segment_sum(gathered, node_idx) # [N, 128]
    mean     = summed / max(count, 1)
    out      = mean @ W.T + b                  # [N, 128]

Sharding: edges are partitioned by destination-node range across the 8
NeuronCores (12500 nodes per core); x_clique and the 128x128 Linear are
replicated. Segment-sum applies locally, no cross-device reduction.

Per-core device algorithm (v2):
  - host sorts edges by destination and buckets them into 98 blocks of
    128 destination nodes; each block's edge list is split by clique id
    at 32768 (dma_gather indices are int16) into an A and a B stream,
    each padded to a fixed tile count (T_A / T_B) with NEGATIVE indices.
    The dma_gather ucode trims trailing negative indices at runtime, so
    the padding costs no descriptor-generation time (the real indices of
    each stream come first and are all >= 0).
  - one dma_gather per (block, stream), 196 per core. dma_gather runs on
    the GpSimd core pair (2q, 2q+1) selected by queue_num; with
    num_swdge_queues=4 and round-robin queue assignment up to 4 gathers
    overlap on disjoint core pairs (measured ~2x+ on HW; descriptor
    generation at ~7.8 ns/row + 535 ns/instruction is the baseline
    bottleneck at 97% GpSimd occupancy).
  - tables are bf16: halves gather payload and doubles PE throughput.
  - a one-hot matrix (edge -> node-within-block) is built with a batched
    DVE is_equal against an iota tile; padding slots miss (dest -1000).
  - PE accumulates accum[f, n] += G[e, f].T @ onehot[e, n] in PSUM; the
    gathered tile must be the STATIONARY operand -- the PE's
    moving-operand path crashes when streaming a dma_gather-written tile.
    Rows of G beyond the trimmed gather are stale SBUF data multiplied by
    a zero one-hot column; buffers are memset once so they are never NaN.
  - epilogue per block: ACT copies PSUM->SBUF, one matmul with W.T
    applies the Linear directly on the [f, n] accumulator, ACT scales by
    1/count, DVE adds the bias, DMA writes 128 rows out.
"""

import os
import sys
import types

sys.path.insert(0, "/opt/trn_rl_repo")

import numpy as np

import concourse.bass as bass
import concourse.mybir as mybir
import concourse.tile as tile
from concourse.vector_clock import ScopedClock, VectorClock
from concourse.bass_utils import run_bass_kernel_spmd

# ----------------------------------------------------------------------------
# Environment shims
# ----------------------------------------------------------------------------

def _install_ntff_shim():
    """Register the axon NTFF profile hook if the image's antenv lacks it."""
    try:
        import antenv
    except ImportError:
        return
    if hasattr(antenv, "axon_hooks"):
        return
    hooks_mod = types.ModuleType("antenv.axon_hooks")
    _store = [None]
    hooks_mod.set_axon_ntff_profile_hook = lambda h: _store.__setitem__(0, h)
    hooks_mod.get_axon_ntff_profile_hook = lambda: _store[0]
    sys.modules["antenv.axon_hooks"] = hooks_mod
    antenv.axon_hooks = hooks_mod
    try:
        from trn_agent_boot.trn_boot import _ntff_profile_via_ctypes

        hook = _ntff_profile_via_ctypes("/opt/axon/libaxon_pjrt.so")
        if hook is not None:
            hooks_mod.set_axon_ntff_profile_hook(hook)
    except Exception:
        pass


_install_ntff_shim()


class PatchedTileContext(tile.TileContext):
    """Spread the tail-drain's sem waits over a chain of SP NOPs.

    The walrus build in this container caps sync-waits per instruction
    (setupSyncWait: "Too many sync wait commands"), while stock Tile
    attaches every outstanding proc's wait to one Drain. One NOP per
    proc keeps every instruction at a single wait.
    """

    def _drain_and_barrier(self, tick_clock, wait_clock):
        gc = tick_clock.global_clock
        for p, t in enumerate(gc):
            if t <= 0:
                continue
            nop = self.nc.sync.nop()
            part = VectorClock()
            part.require_at_least(p, t)
            wait_clock.add_sem_waits(nop.ins, ScopedClock({None: part}))
        self.nc.sync.drain()
        self.nc.all_engine_barrier()
        assert self.sems is not None
        popped = self.nc._tile_sem_poison_stack.pop()
        assert popped is self._sem_poison
        self.nc.clear_and_free_semaphores(list(self.sems.allocated().values()))
        self.nc.all_engine_barrier()


# ----------------------------------------------------------------------------
# Problem constants (hardcoded per the task contract)
# ----------------------------------------------------------------------------

N_NODES = 100000
N_CLIQUES = 50000
D = 128
N_CORES = 8
NPC = N_NODES // N_CORES        # 12500 nodes per core
BLK = 128                       # destination nodes per block
NBLK = -(-NPC // BLK)           # 98 blocks per core (last partial: 84)
NPAD = NBLK * BLK               # 12544 padded output rows per core
SPLIT = 32768                   # int16-index limit for dma_gather
PAD_DEST = -1000.0              # one-hot miss value for padding slots

# f32 tables cost nothing on descgen but double DMA bytes and PE time;
# bf16 keeps rel-err ~4e-3 (gate 2e-2). Default bf16, f32 via env.
USE_BF16 = os.environ.get("KERNEL_BF16", "1") == "1"

# SWDGE queues: each dma_gather runs on GpSimd core pair (2q, 2q+1); with
# NQ>1 consecutive gathers go to different pairs and overlap on HW.
NQ = int(os.environ.get("KERNEL_NQ", "4"))
GBUFS = int(os.environ.get("KERNEL_GBUFS", "6"))
# 1: pad to the 128-chunk boundary with index 0, -1 beyond (ucode trims the
# trailing negatives). 0: fill all padding with index 0 (no trim).
TRIM = int(os.environ.get("KERNEL_TRIM", "1"))

_F32 = mybir.dt.float32
_DT = mybir.dt.bfloat16 if USE_BF16 else _F32

if USE_BF16:
    import ml_dtypes

    _NP_DT = np.dtype(ml_dtypes.bfloat16)
else:
    _NP_DT = np.dtype(np.float32)


# ----------------------------------------------------------------------------
# Host-side preparation
# ----------------------------------------------------------------------------

def _prepare(x_clique, node2clique_index):
    """Sort/bucket/pad the edge list. Returns per-core input dicts plus the
    (data-dependent) tile counts T_A, T_B."""
    node = np.asarray(node2clique_index[0]).astype(np.int64)
    clique = np.asarray(node2clique_index[1]).astype(np.int64)

    counts = np.bincount(node, minlength=N_NODES).astype(np.float64)
    inv_cnt = (1.0 / np.maximum(counts, 1.0)).astype(np.float32)

    order = np.argsort(node, kind="stable")
    ns = node[order]
    cs = clique[order]

    core_bounds = np.searchsorted(ns, np.arange(N_CORES + 1) * NPC)

    # First pass: per-(core, block) A/B counts to fix the global T_A, T_B.
    per_core = []
    maxA = 0
    maxB = 0
    for c in range(N_CORES):
        lo, hi = core_bounds[c], core_bounds[c + 1]
        loc = ns[lo:hi] - c * NPC
        cq = cs[lo:hi]
        blk = loc // BLK
        win = loc % BLK
        is_a = cq < SPLIT
        # edges already sorted by loc; stable-partition A before B per block
        key = blk * 2 + (~is_a)
        sub = np.argsort(key, kind="stable")
        blk, win, cq, is_a = blk[sub], win[sub], cq[sub], is_a[sub]
        cntA = np.bincount(blk[is_a], minlength=NBLK)
        cntB = np.bincount(blk[~is_a], minlength=NBLK)
        maxA = max(maxA, int(cntA.max()))
        maxB = max(maxB, int(cntB.max()))
        per_core.append((blk, win, cq, is_a, cntA, cntB))

    T_A = -(-maxA // 128)
    T_B = max(-(-maxB // 128), 1)
    T = T_A + T_B
    LA = T_A * 128
    LB = T_B * 128
    np_dt = _NP_DT

    in_maps = []
    for c in range(N_CORES):
        blk, win, cq, is_a, cntA, cntB = per_core[c]

        # -1 padding: the dma_gather ucode trims trailing negative indices,
        # so per-block padding costs no descriptor generation. Real indices
        # are padded with 0 up to the next 128-chunk boundary first, so the
        # trimmed count is always a multiple of 128 and the descriptor
        # generator never sees a partially-valid lane group (untested ucode
        # path on this build).
        idxA = np.full((NBLK, LA), -1, dtype=np.int16)
        idxB = np.full((NBLK, LB), -1, dtype=np.int16)
        dest = np.full((NBLK, T * 128), PAD_DEST, dtype=np.float32)

        offA = np.concatenate([[0], np.cumsum(cntA)])
        offB = np.concatenate([[0], np.cumsum(cntB)])

        a_idx = np.flatnonzero(is_a)
        b_idx = np.flatnonzero(~is_a)
        cqA, winA, blkA = cq[a_idx], win[a_idx], blk[a_idx]
        cqB, winB, blkB = cq[b_idx] - SPLIT, win[b_idx], blk[b_idx]

        posA = np.arange(len(a_idx)) - offA[blkA]
        posB = np.arange(len(b_idx)) - offB[blkB]
        idxA[blkA, posA] = cqA.astype(np.int16)
        idxB[blkB, posB] = cqB.astype(np.int16)
        dest[blkA, posA] = winA
        dest[blkB, posB + LA] = winB

        # pad each block's real indices with 0 (a real gather of row 0;
        # dest stays PAD -> zero one-hot) up to the next 128 multiple;
        # -1 beyond that is trimmed. Also covers empty blocks (cnt=0 ->
        # one full chunk of index 0).
        if TRIM:
            up_a = np.minimum(-(-np.maximum(cntA, 1) // 128) * 128, LA)
            up_b = np.minimum(-(-np.maximum(cntB, 1) // 128) * 128, LB)
        else:
            up_a = np.full(NBLK, LA)
            up_b = np.full(NBLK, LB)
        colA = np.arange(LA)[None, :]
        colB = np.arange(LB)[None, :]
        fillA = (idxA == -1) & (colA < up_a[:, None])
        fillB = (idxB == -1) & (colB < up_b[:, None])
        idxA[fillA] = 0
        idxB[fillB] = 0

        # runtime valid-index counts, one per gather: the decode stage
        # reserves ring slots from num_idxs_reg while the Q7 ucode counts
        # the trimmed indices -- the two MUST match or the ring desyncs
        # and the device hangs.
        cnts = np.empty((1, 2 * NBLK), dtype=np.int32)
        cnts[0, 0::2] = up_a
        cnts[0, 1::2] = up_b

        # wrap indices for dma_gather: seq j -> [j % 16, j // 16], one block
        # per gather call. dma_gather reads a [128, n/16] idx AP: the
        # [16, n/16] wrap is replicated across all 8 GpSimd cores'
        # partition groups.
        def _wrap(idx, L):
            w = idx.reshape(NBLK, -1, 16)
            w = np.ascontiguousarray(np.transpose(w, (2, 0, 1))).reshape(16, -1)
            return np.tile(w, (8, 1))

        wA = _wrap(idxA, LA)
        wB = _wrap(idxB, LB)

        # dest layout for the batched is_equal: [128, NBLK * T]
        dest_t = np.ascontiguousarray(
            dest.reshape(NBLK * T, 128).T
        ).astype(np_dt)

        inv_t = np.zeros((BLK, NBLK), dtype=np.float32)
        iv = inv_cnt[c * NPC : (c + 1) * NPC]
        inv_t.T.flat[: NPC] = iv  # row-major [NBLK, BLK] view fill
        inv_t = np.ascontiguousarray(inv_t)

        in_maps.append(
            {
                "idxA": wA,
                "idxB": wB,
                "dest": dest_t,
                "invc": inv_t,
                "cnts": cnts,
            }
        )

    shared = {
        "xcA": np.ascontiguousarray(np.asarray(x_clique)[:SPLIT]).astype(np_dt),
        "xcB": np.ascontiguousarray(np.asarray(x_clique)[SPLIT:]).astype(np_dt),
        "iota": np.tile(np.arange(128, dtype=np.float32), (128, 1)).astype(np_dt),
    }
    return in_maps, shared, T_A, T_B


# ----------------------------------------------------------------------------
# Kernel builder
# ----------------------------------------------------------------------------

def _build(T_A, T_B):
    T = T_A + T_B
    LA, LB = T_A * 128, T_B * 128
    CB = N_CLIQUES - SPLIT

    from concourse.bacc import Bacc

    nc = Bacc(None, num_swdge_queues=NQ)
    xcA = nc.declare_dram_parameter("xcA", [SPLIT, D], _DT, isOutput=False)
    xcB = nc.declare_dram_parameter("xcB", [CB, D], _DT, isOutput=False)
    idxA = nc.declare_dram_parameter(
        "idxA", [128, NBLK * LA // 16], mybir.dt.int16, isOutput=False
    )
    idxB = nc.declare_dram_parameter(
        "idxB", [128, NBLK * LB // 16], mybir.dt.int16, isOutput=False
    )
    dest = nc.declare_dram_parameter("dest", [128, NBLK * T], _DT, isOutput=False)
    invc = nc.declare_dram_parameter("invc", [128, NBLK], _F32, isOutput=False)
    cnts = nc.declare_dram_parameter(
        "cnts", [1, 2 * NBLK], mybir.dt.int32, isOutput=False
    )
    iota = nc.declare_dram_parameter("iota", [128, 128], _DT, isOutput=False)
    wt = nc.declare_dram_parameter("wt", [128, 128], _DT, isOutput=False)
    bb = nc.declare_dram_parameter("bb", [128, 128], _F32, isOutput=False)
    out = nc.declare_dram_parameter("out", [NPAD, D], _F32, isOutput=True)

    from contextlib import ExitStack

    with PatchedTileContext(nc) as tc, ExitStack() as ctx:
        const = ctx.enter_context(tc.tile_pool(name="const", bufs=1))
        sb = ctx.enter_context(tc.tile_pool(name="sb", bufs=3))
        gpool = ctx.enter_context(tc.tile_pool(name="g", bufs=GBUFS))
        ps = ctx.enter_context(tc.tile_pool(name="ps", bufs=2, space="PSUM"))

        # idx tables are loaded in block-range chunks as separate tiles so
        # the first gathers only wait for their own slice, not the full
        # ~4MB of index data.
        IDX_CHUNK = 14
        NCH = -(-NBLK // IDX_CHUNK)
        idxA_ts = []
        idxB_ts = []
        for k in range(NCH):
            blo = k * IDX_CHUNK
            bhi = min(NBLK, blo + IDX_CHUNK)
            ta = const.tile([128, (bhi - blo) * (LA // 16)], mybir.dt.int16)
            nc.sync.dma_start(
                ta[:], idxA[:, blo * (LA // 16) : bhi * (LA // 16)]
            )
            idxA_ts.append(ta)
            tb = const.tile([128, (bhi - blo) * (LB // 16)], mybir.dt.int16)
            nc.sync.dma_start(
                tb[:], idxB[:, blo * (LB // 16) : bhi * (LB // 16)]
            )
            idxB_ts.append(tb)

        def idxA_slice(b):
            k, r = divmod(b, IDX_CHUNK)
            return idxA_ts[k][:, r * (LA // 16) : (r + 1) * (LA // 16)]

        def idxB_slice(b):
            k, r = divmod(b, IDX_CHUNK)
            return idxB_ts[k][:, r * (LB // 16) : (r + 1) * (LB // 16)]
        dest_t = const.tile([128, NBLK * T], _DT)
        nc.sync.dma_start(dest_t[:], dest[:])
        invc_t = const.tile([128, NBLK], _F32)
        nc.sync.dma_start(invc_t[:], invc[:])
        cnts_t = const.tile([1, 2 * NBLK], mybir.dt.int32)
        nc.sync.dma_start(cnts_t[:], cnts[:])
        iota_t = const.tile([128, 128], _DT)
        nc.sync.dma_start(iota_t[:], iota[:])
        wt_t = const.tile([128, 128], _DT)
        nc.sync.dma_start(wt_t[:], wt[:])
        bb_t = const.tile([128, 128], _F32)
        nc.sync.dma_start(bb_t[:], bb[:])

        for b in range(NBLK):
            gA = gpool.tile([128, T_A, 128], _DT, tag="gA")
            gB = gpool.tile([128, T_B, 128], _DT, tag="gB")
            if b < GBUFS:
                # rows past the runtime-trimmed gather stay stale in SBUF;
                # zero each physical buffer once so they are never NaN
                # (stale values only ever meet zero one-hot columns).
                nc.vector.memset(gA[:], 0.0)
                nc.vector.memset(gB[:], 0.0)
            if TRIM:
                cA = nc.gpsimd.value_load(cnts_t[0:1, 2 * b : 2 * b + 1])
                cB = nc.gpsimd.value_load(cnts_t[0:1, 2 * b + 1 : 2 * b + 2])
            else:
                cA, cB = LA, LB
            nc.gpsimd.dma_gather(
                gA[:],
                xcA[:],
                idxA_slice(b),
                LA,
                cA,
                D,
                single_packet=False,
                queue_num=(3 * b) % NQ if NQ > 1 else 0,
            )
            nc.gpsimd.dma_gather(
                gB[:],
                xcB[:],
                idxB_slice(b),
                LB,
                cB,
                D,
                single_packet=False,
                queue_num=(3 * b + 1) % NQ if NQ > 1 else 0,
            )
            onehot = sb.tile([128, T, 128], _DT, tag="oh")
            nc.vector.tensor_tensor(
                out=onehot[:],
                in0=dest_t[:, b * T : (b + 1) * T, None].to_broadcast(
                    [128, T, 128]
                ),
                in1=iota_t[:, None, :].to_broadcast([128, T, 128]),
                op=mybir.AluOpType.is_equal,
            )
            # accum[f, n] += G[e, f].T @ onehot[e, n] -- the gathered tile
            # must be the STATIONARY operand (LDWEIGHTS path); the moving
            # path crashes the PE when reading a dma_gather-written tile.
            accum = ps.tile([128, 128], _F32, tag="acc")
            for t in range(T_A):
                nc.tensor.matmul(
                    out=accum[:],
                    lhsT=gA[:, t, :],
                    rhs=onehot[:, t, :],
                    start=(t == 0),
                    stop=False,
                )
            for t in range(T_B):
                nc.tensor.matmul(
                    out=accum[:],
                    lhsT=gB[:, t, :],
                    rhs=onehot[:, T_A + t, :],
                    start=False,
                    stop=(t == T_B - 1),
                )
            # accum is summed.T -- exactly the lhsT the Linear wants.
            acc_sb = sb.tile([128, 128], _DT, tag="accsb")
            nc.scalar.activation(
                acc_sb[:], accum[:], mybir.ActivationFunctionType.Copy
            )
            lin = ps.tile([128, 128], _F32, tag="lin")
            nc.tensor.matmul(
                out=lin[:], lhsT=acc_sb[:], rhs=wt_t[:], start=True, stop=True
            )
            # out[n, o] = lin[n, o] / count[n] + b[o]
            sc = sb.tile([128, 128], _F32, tag="sc")
            nc.scalar.activation(
                sc[:],
                lin[:],
                mybir.ActivationFunctionType.Copy,
                scale=invc_t[:, b : b + 1],
            )
            outs = sb.tile([128, 128], _F32, tag="outs")
            nc.vector.tensor_tensor(
                out=outs[:], in0=sc[:], in1=bb_t[:], op=mybir.AluOpType.add
            )
            nc.sync.dma_start(out[b * 128 : (b + 1) * 128, :], outs[:])

    nc.finalize()
    return nc


_BUILD_CACHE = {}


def kernel(x, x_clique, node2clique_index, W, b, _trace=False, _tmpdir=None):
    in_maps, shared, T_A, T_B = _prepare(x_clique, node2clique_index)

    shared["wt"] = np.ascontiguousarray(np.asarray(W, dtype=np.float32).T).astype(
        _NP_DT
    )
    shared["bb"] = np.tile(
        np.asarray(b, dtype=np.float32)[None, :], (128, 1)
    ).astype(np.float32)

    key = (T_A, T_B, USE_BF16, NQ, GBUFS, TRIM)
    if key not in _BUILD_CACHE:
        _BUILD_CACHE[key] = _build(T_A, T_B)
    nc = _BUILD_CACHE[key]

    full_maps = [dict(m, **shared) for m in in_maps]
    kwargs = {}
    if _trace:
        kwargs = dict(trace=True, tmpdir=_tmpdir)
    res = run_bass_kernel_spmd(nc, full_maps, core_ids=list(range(N_CORES)), **kwargs)

    out = np.concatenate(
        [res.results[c]["out"][:NPC] for c in range(N_CORES)], axis=0
    ).astype(np.float32)
    if _trace:
        return out, res
    return out


# revision 26
# speedup vs baseline: 1.0352x; 1.0352x over previous
"""Trainium2 Bass kernel for Clique2NodeConvBasic (GNN message passing).

Computes, for the fixed problem size N=100000 nodes, C=50000 cliques,
E=1600000 edges, D=128:

    gathered = x_clique[clique_idx]            # [E, 128]
    summed   = segment_sum(gathered, node_idx) # [N, 128]
    mean     = summed / max(count, 1)
    out      = mean @ W.T + b                  # [N, 128]

Sharding: edges are partitioned by destination-node range across the 8
NeuronCores (12500 nodes per core); x_clique and the 128x128 Linear are
replicated. Segment-sum applies locally, no cross-device reduction.

Per-core device algorithm (v2):
  - host sorts edges by destination and buckets them into 98 blocks of
    128 destination nodes; each block's edge list is split by clique id
    at 32768 (dma_gather indices are int16) into an A and a B stream,
    each padded to a fixed tile count (T_A / T_B) with NEGATIVE indices.
    The dma_gather ucode trims trailing negative indices at runtime, so
    the padding costs no descriptor-generation time (the real indices of
    each stream come first and are all >= 0).
  - one dma_gather per (block, stream), 196 per core. dma_gather runs on
    the GpSimd core pair (2q, 2q+1) selected by queue_num; with
    num_swdge_queues=4 and round-robin queue assignment up to 4 gathers
    overlap on disjoint core pairs (measured ~2x+ on HW; descriptor
    generation at ~7.8 ns/row + 535 ns/instruction is the baseline
    bottleneck at 97% GpSimd occupancy).
  - tables are bf16: halves gather payload and doubles PE throughput.
  - a one-hot matrix (edge -> node-within-block) is built with a batched
    DVE is_equal against an iota tile; padding slots miss (dest -1000).
  - PE accumulates accum[f, n] += G[e, f].T @ onehot[e, n] in PSUM; the
    gathered tile must be the STATIONARY operand -- the PE's
    moving-operand path crashes when streaming a dma_gather-written tile.
    Rows of G beyond the trimmed gather are stale SBUF data multiplied by
    a zero one-hot column; buffers are memset once so they are never NaN.
  - epilogue per block: ACT copies PSUM->SBUF, one matmul with W.T
    applies the Linear directly on the [f, n] accumulator, ACT scales by
    1/count, DVE adds the bias, DMA writes 128 rows out.
"""

import os
import sys
import types

sys.path.insert(0, "/opt/trn_rl_repo")

import numpy as np

import concourse.bass as bass
import concourse.mybir as mybir
import concourse.tile as tile
from concourse.vector_clock import ScopedClock, VectorClock
from concourse.bass_utils import run_bass_kernel_spmd

# ----------------------------------------------------------------------------
# Environment shims
# ----------------------------------------------------------------------------

def _install_ntff_shim():
    """Register the axon NTFF profile hook if the image's antenv lacks it."""
    try:
        import antenv
    except ImportError:
        return
    if hasattr(antenv, "axon_hooks"):
        return
    hooks_mod = types.ModuleType("antenv.axon_hooks")
    _store = [None]
    hooks_mod.set_axon_ntff_profile_hook = lambda h: _store.__setitem__(0, h)
    hooks_mod.get_axon_ntff_profile_hook = lambda: _store[0]
    sys.modules["antenv.axon_hooks"] = hooks_mod
    antenv.axon_hooks = hooks_mod
    try:
        from trn_agent_boot.trn_boot import _ntff_profile_via_ctypes

        hook = _ntff_profile_via_ctypes("/opt/axon/libaxon_pjrt.so")
        if hook is not None:
            hooks_mod.set_axon_ntff_profile_hook(hook)
    except Exception:
        pass


_install_ntff_shim()


class PatchedTileContext(tile.TileContext):
    """Spread the tail-drain's sem waits over a chain of SP NOPs.

    The walrus build in this container caps sync-waits per instruction
    (setupSyncWait: "Too many sync wait commands"), while stock Tile
    attaches every outstanding proc's wait to one Drain. One NOP per
    proc keeps every instruction at a single wait.
    """

    def _drain_and_barrier(self, tick_clock, wait_clock):
        gc = tick_clock.global_clock
        for p, t in enumerate(gc):
            if t <= 0:
                continue
            nop = self.nc.sync.nop()
            part = VectorClock()
            part.require_at_least(p, t)
            wait_clock.add_sem_waits(nop.ins, ScopedClock({None: part}))
        self.nc.sync.drain()
        self.nc.all_engine_barrier()
        assert self.sems is not None
        popped = self.nc._tile_sem_poison_stack.pop()
        assert popped is self._sem_poison
        self.nc.clear_and_free_semaphores(list(self.sems.allocated().values()))
        self.nc.all_engine_barrier()


# ----------------------------------------------------------------------------
# Problem constants (hardcoded per the task contract)
# ----------------------------------------------------------------------------

N_NODES = 100000
N_CLIQUES = 50000
D = 128
N_CORES = 8
NPC = N_NODES // N_CORES        # 12500 nodes per core
BLK = 128                       # destination nodes per block
NBLK = -(-NPC // BLK)           # 98 blocks per core (last partial: 84)
NPAD = NBLK * BLK               # 12544 padded output rows per core
SPLIT = 32768                   # int16-index limit for dma_gather
PAD_DEST = -1000.0              # one-hot miss value for padding slots

# f32 tables cost nothing on descgen but double DMA bytes and PE time;
# bf16 keeps rel-err ~4e-3 (gate 2e-2). Default bf16, f32 via env.
USE_BF16 = os.environ.get("KERNEL_BF16", "1") == "1"

# SWDGE queues: each dma_gather runs on GpSimd core pair (2q, 2q+1); with
# NQ>1 consecutive gathers go to different pairs and overlap on HW.
NQ = int(os.environ.get("KERNEL_NQ", "4"))
GBUFS = int(os.environ.get("KERNEL_GBUFS", "6"))
# 1: pad to the 128-chunk boundary with index 0, -1 beyond (ucode trims the
# trailing negatives). 0: fill all padding with index 0 (no trim).
TRIM = int(os.environ.get("KERNEL_TRIM", "1"))
# single_packet=True coalesces each ring's gather stream into one DMA packet
# (first/concatenate/last framing), cutting per-packet completion overhead.
SP = os.environ.get("KERNEL_SP", "0") == "1"

_F32 = mybir.dt.float32
_DT = mybir.dt.bfloat16 if USE_BF16 else _F32

if USE_BF16:
    import ml_dtypes

    _NP_DT = np.dtype(ml_dtypes.bfloat16)
else:
    _NP_DT = np.dtype(np.float32)


# ----------------------------------------------------------------------------
# Host-side preparation
# ----------------------------------------------------------------------------

def _prepare(x_clique, node2clique_index):
    """Sort/bucket/pad the edge list. Returns per-core input dicts plus the
    (data-dependent) tile counts T_A, T_B."""
    node = np.asarray(node2clique_index[0]).astype(np.int64)
    clique = np.asarray(node2clique_index[1]).astype(np.int64)

    counts = np.bincount(node, minlength=N_NODES).astype(np.float64)
    inv_cnt = (1.0 / np.maximum(counts, 1.0)).astype(np.float32)

    order = np.argsort(node, kind="stable")
    ns = node[order]
    cs = clique[order]

    core_bounds = np.searchsorted(ns, np.arange(N_CORES + 1) * NPC)

    # First pass: per-(core, block) A/B counts to fix the global T_A, T_B.
    per_core = []
    maxA = 0
    maxB = 0
    for c in range(N_CORES):
        lo, hi = core_bounds[c], core_bounds[c + 1]
        loc = ns[lo:hi] - c * NPC
        cq = cs[lo:hi]
        blk = loc // BLK
        win = loc % BLK
        is_a = cq < SPLIT
        # edges already sorted by loc; stable-partition A before B per block
        key = blk * 2 + (~is_a)
        sub = np.argsort(key, kind="stable")
        blk, win, cq, is_a = blk[sub], win[sub], cq[sub], is_a[sub]
        cntA = np.bincount(blk[is_a], minlength=NBLK)
        cntB = np.bincount(blk[~is_a], minlength=NBLK)
        maxA = max(maxA, int(cntA.max()))
        maxB = max(maxB, int(cntB.max()))
        per_core.append((blk, win, cq, is_a, cntA, cntB))

    T_A = -(-maxA // 128)
    T_B = max(-(-maxB // 128), 1)
    T = T_A + T_B
    LA = T_A * 128
    LB = T_B * 128
    np_dt = _NP_DT

    in_maps = []
    for c in range(N_CORES):
        blk, win, cq, is_a, cntA, cntB = per_core[c]

        # -1 padding: the dma_gather ucode trims trailing negative indices,
        # so per-block padding costs no descriptor generation. Real indices
        # are padded with 0 up to the next 128-chunk boundary first, so the
        # trimmed count is always a multiple of 128 and the descriptor
        # generator never sees a partially-valid lane group (untested ucode
        # path on this build).
        idxA = np.full((NBLK, LA), -1, dtype=np.int16)
        idxB = np.full((NBLK, LB), -1, dtype=np.int16)
        dest = np.full((NBLK, T * 128), PAD_DEST, dtype=np.float32)

        offA = np.concatenate([[0], np.cumsum(cntA)])
        offB = np.concatenate([[0], np.cumsum(cntB)])

        a_idx = np.flatnonzero(is_a)
        b_idx = np.flatnonzero(~is_a)
        cqA, winA, blkA = cq[a_idx], win[a_idx], blk[a_idx]
        cqB, winB, blkB = cq[b_idx] - SPLIT, win[b_idx], blk[b_idx]

        posA = np.arange(len(a_idx)) - offA[blkA]
        posB = np.arange(len(b_idx)) - offB[blkB]
        idxA[blkA, posA] = cqA.astype(np.int16)
        idxB[blkB, posB] = cqB.astype(np.int16)
        dest[blkA, posA] = winA
        dest[blkB, posB + LA] = winB

        # pad each block's real indices with 0 (a real gather of row 0;
        # dest stays PAD -> zero one-hot) up to the next 128 multiple;
        # -1 beyond that is trimmed. Also covers empty blocks (cnt=0 ->
        # one full chunk of index 0).
        if TRIM:
            up_a = np.minimum(-(-np.maximum(cntA, 1) // 128) * 128, LA)
            up_b = np.minimum(-(-np.maximum(cntB, 1) // 128) * 128, LB)
        else:
            up_a = np.full(NBLK, LA)
            up_b = np.full(NBLK, LB)
        colA = np.arange(LA)[None, :]
        colB = np.arange(LB)[None, :]
        fillA = (idxA == -1) & (colA < up_a[:, None])
        fillB = (idxB == -1) & (colB < up_b[:, None])
        idxA[fillA] = 0
        idxB[fillB] = 0

        # runtime valid-index counts, one per gather: the decode stage
        # reserves ring slots from num_idxs_reg while the Q7 ucode counts
        # the trimmed indices -- the two MUST match or the ring desyncs
        # and the device hangs.
        cnts = np.empty((1, 2 * NBLK), dtype=np.int32)
        cnts[0, 0::2] = up_a
        cnts[0, 1::2] = up_b

        # wrap indices for dma_gather: seq j -> [j % 16, j // 16], one block
        # per gather call. dma_gather reads a [128, n/16] idx AP: the
        # [16, n/16] wrap is replicated across all 8 GpSimd cores'
        # partition groups.
        def _wrap(idx, L):
            w = idx.reshape(NBLK, -1, 16)
            w = np.ascontiguousarray(np.transpose(w, (2, 0, 1))).reshape(16, -1)
            return np.tile(w, (8, 1))

        wA = _wrap(idxA, LA)
        wB = _wrap(idxB, LB)

        # dest layout for the batched is_equal: [128, NBLK * T]
        dest_t = np.ascontiguousarray(
            dest.reshape(NBLK * T, 128).T
        ).astype(np_dt)

        inv_t = np.zeros((BLK, NBLK), dtype=np.float32)
        iv = inv_cnt[c * NPC : (c + 1) * NPC]
        inv_t.T.flat[: NPC] = iv  # row-major [NBLK, BLK] view fill
        inv_t = np.ascontiguousarray(inv_t)

        in_maps.append(
            {
                "idxA": wA,
                "idxB": wB,
                "dest": dest_t,
                "invc": inv_t,
                "cnts": cnts,
            }
        )

    shared = {
        "xcA": np.ascontiguousarray(np.asarray(x_clique)[:SPLIT]).astype(np_dt),
        "xcB": np.ascontiguousarray(np.asarray(x_clique)[SPLIT:]).astype(np_dt),
        "iota": np.tile(np.arange(128, dtype=np.float32), (128, 1)).astype(np_dt),
    }
    return in_maps, shared, T_A, T_B


# ----------------------------------------------------------------------------
# Kernel builder
# ----------------------------------------------------------------------------

def _build(T_A, T_B):
    T = T_A + T_B
    LA, LB = T_A * 128, T_B * 128
    CB = N_CLIQUES - SPLIT

    from concourse.bacc import Bacc

    nc = Bacc(None, num_swdge_queues=NQ)
    xcA = nc.declare_dram_parameter("xcA", [SPLIT, D], _DT, isOutput=False)
    xcB = nc.declare_dram_parameter("xcB", [CB, D], _DT, isOutput=False)
    idxA = nc.declare_dram_parameter(
        "idxA", [128, NBLK * LA // 16], mybir.dt.int16, isOutput=False
    )
    idxB = nc.declare_dram_parameter(
        "idxB", [128, NBLK * LB // 16], mybir.dt.int16, isOutput=False
    )
    dest = nc.declare_dram_parameter("dest", [128, NBLK * T], _DT, isOutput=False)
    invc = nc.declare_dram_parameter("invc", [128, NBLK], _F32, isOutput=False)
    cnts = nc.declare_dram_parameter(
        "cnts", [1, 2 * NBLK], mybir.dt.int32, isOutput=False
    )
    iota = nc.declare_dram_parameter("iota", [128, 128], _DT, isOutput=False)
    wt = nc.declare_dram_parameter("wt", [128, 128], _DT, isOutput=False)
    bb = nc.declare_dram_parameter("bb", [128, 128], _F32, isOutput=False)
    out = nc.declare_dram_parameter("out", [NPAD, D], _F32, isOutput=True)

    from contextlib import ExitStack

    with PatchedTileContext(nc) as tc, ExitStack() as ctx:
        const = ctx.enter_context(tc.tile_pool(name="const", bufs=1))
        sb = ctx.enter_context(tc.tile_pool(name="sb", bufs=3))
        gpool = ctx.enter_context(tc.tile_pool(name="g", bufs=GBUFS))
        ps = ctx.enter_context(tc.tile_pool(name="ps", bufs=2, space="PSUM"))

        idxA_t = const.tile([128, NBLK * LA // 16], mybir.dt.int16)
        nc.sync.dma_start(idxA_t[:], idxA[:])
        idxB_t = const.tile([128, NBLK * LB // 16], mybir.dt.int16)
        nc.sync.dma_start(idxB_t[:], idxB[:])

        def idxA_slice(b):
            return idxA_t[:, b * (LA // 16) : (b + 1) * (LA // 16)]

        def idxB_slice(b):
            return idxB_t[:, b * (LB // 16) : (b + 1) * (LB // 16)]
        dest_t = const.tile([128, NBLK * T], _DT)
        nc.sync.dma_start(dest_t[:], dest[:])
        invc_t = const.tile([128, NBLK], _F32)
        nc.sync.dma_start(invc_t[:], invc[:])
        cnts_t = const.tile([1, 2 * NBLK], mybir.dt.int32)
        nc.sync.dma_start(cnts_t[:], cnts[:])
        iota_t = const.tile([128, 128], _DT)
        nc.sync.dma_start(iota_t[:], iota[:])
        wt_t = const.tile([128, 128], _DT)
        nc.sync.dma_start(wt_t[:], wt[:])
        bb_t = const.tile([128, 128], _F32)
        nc.sync.dma_start(bb_t[:], bb[:])

        for b in range(NBLK):
            gA = gpool.tile([128, T_A, 128], _DT, tag="gA")
            gB = gpool.tile([128, T_B, 128], _DT, tag="gB")
            if b < GBUFS:
                # rows past the runtime-trimmed gather stay stale in SBUF;
                # zero each physical buffer once so they are never NaN
                # (stale values only ever meet zero one-hot columns).
                nc.vector.memset(gA[:], 0.0)
                nc.vector.memset(gB[:], 0.0)
            if TRIM:
                cA = nc.gpsimd.value_load(cnts_t[0:1, 2 * b : 2 * b + 1])
                cB = nc.gpsimd.value_load(cnts_t[0:1, 2 * b + 1 : 2 * b + 2])
            else:
                cA, cB = LA, LB
            nc.gpsimd.dma_gather(
                gA[:],
                xcA[:],
                idxA_slice(b),
                LA,
                cA,
                D,
                single_packet=SP,
                queue_num=(3 * b) % NQ if NQ > 1 else 0,
            )
            nc.gpsimd.dma_gather(
                gB[:],
                xcB[:],
                idxB_slice(b),
                LB,
                cB,
                D,
                single_packet=SP,
                queue_num=(3 * b + 1) % NQ if NQ > 1 else 0,
            )
            onehot = sb.tile([128, T, 128], _DT, tag="oh")
            nc.vector.tensor_tensor(
                out=onehot[:],
                in0=dest_t[:, b * T : (b + 1) * T, None].to_broadcast(
                    [128, T, 128]
                ),
                in1=iota_t[:, None, :].to_broadcast([128, T, 128]),
                op=mybir.AluOpType.is_equal,
            )
            # accum[f, n] += G[e, f].T @ onehot[e, n] -- the gathered tile
            # must be the STATIONARY operand (LDWEIGHTS path); the moving
            # path crashes the PE when reading a dma_gather-written tile.
            accum = ps.tile([128, 128], _F32, tag="acc")
            for t in range(T_A):
                nc.tensor.matmul(
                    out=accum[:],
                    lhsT=gA[:, t, :],
                    rhs=onehot[:, t, :],
                    start=(t == 0),
                    stop=False,
                )
            for t in range(T_B):
                nc.tensor.matmul(
                    out=accum[:],
                    lhsT=gB[:, t, :],
                    rhs=onehot[:, T_A + t, :],
                    start=False,
                    stop=(t == T_B - 1),
                )
            # accum is summed.T -- exactly the lhsT the Linear wants.
            acc_sb = sb.tile([128, 128], _DT, tag="accsb")
            nc.scalar.activation(
                acc_sb[:], accum[:], mybir.ActivationFunctionType.Copy
            )
            lin = ps.tile([128, 128], _F32, tag="lin")
            nc.tensor.matmul(
                out=lin[:], lhsT=acc_sb[:], rhs=wt_t[:], start=True, stop=True
            )
            # out[n, o] = lin[n, o] / count[n] + b[o]
            sc = sb.tile([128, 128], _F32, tag="sc")
            nc.scalar.activation(
                sc[:],
                lin[:],
                mybir.ActivationFunctionType.Copy,
                scale=invc_t[:, b : b + 1],
            )
            outs = sb.tile([128, 128], _F32, tag="outs")
            nc.vector.tensor_tensor(
                out=outs[:], in0=sc[:], in1=bb_t[:], op=mybir.AluOpType.add
            )
            nc.sync.dma_start(out[b * 128 : (b + 1) * 128, :], outs[:])

    nc.finalize()
    return nc


_BUILD_CACHE = {}


def kernel(x, x_clique, node2clique_index, W, b, _trace=False, _tmpdir=None):
    in_maps, shared, T_A, T_B = _prepare(x_clique, node2clique_index)

    shared["wt"] = np.ascontiguousarray(np.asarray(W, dtype=np.float32).T).astype(
        _NP_DT
    )
    shared["bb"] = np.tile(
        np.asarray(b, dtype=np.float32)[None, :], (128, 1)
    ).astype(np.float32)

    key = (T_A, T_B, USE_BF16, NQ, GBUFS, TRIM, SP)
    if key not in _BUILD_CACHE:
        _BUILD_CACHE[key] = _build(T_A, T_B)
    nc = _BUILD_CACHE[key]

    full_maps = [dict(m, **shared) for m in in_maps]
    kwargs = {}
    if _trace:
        kwargs = dict(trace=True, tmpdir=_tmpdir)
    res = run_bass_kernel_spmd(nc, full_maps, core_ids=list(range(N_CORES)), **kwargs)

    out = np.concatenate(
        [res.results[c]["out"][:NPC] for c in range(N_CORES)], axis=0
    ).astype(np.float32)
    if _trace:
        return out, res
    return out


# revision 36
# speedup vs baseline: 1.1691x; 1.1293x over previous
"""Trainium2 Bass kernel for Clique2NodeConvBasic (GNN message passing).

Computes, for the fixed problem size N=100000 nodes, C=50000 cliques,
E=1600000 edges, D=128:

    gathered = x_clique[clique_idx]            # [E, 128]
    summed   = segment_sum(gathered, node_idx) # [N, 128]
    mean     = summed / max(count, 1)
    out      = mean @ W.T + b                  # [N, 128]

Sharding: edges are partitioned by destination-node range across the 8
NeuronCores (12500 nodes per core); x_clique and the 128x128 Linear are
replicated. Segment-sum applies locally, no cross-device reduction.

Per-core device algorithm (v2):
  - host sorts edges by destination and buckets them into 98 blocks of
    128 destination nodes; each block's edge list is split by clique id
    at 32768 (dma_gather indices are int16) into an A and a B stream,
    each padded to a fixed tile count (T_A / T_B) with NEGATIVE indices.
    The dma_gather ucode trims trailing negative indices at runtime, so
    the padding costs no descriptor-generation time (the real indices of
    each stream come first and are all >= 0).
  - one dma_gather per (block, stream), 196 per core. dma_gather runs on
    the GpSimd core pair (2q, 2q+1) selected by queue_num; with
    num_swdge_queues=4 and round-robin queue assignment up to 4 gathers
    overlap on disjoint core pairs (measured ~2x+ on HW; descriptor
    generation at ~7.8 ns/row + 535 ns/instruction is the baseline
    bottleneck at 97% GpSimd occupancy).
  - tables are bf16: halves gather payload and doubles PE throughput.
  - a one-hot matrix (edge -> node-within-block) is built with a batched
    DVE is_equal against an iota tile; padding slots miss (dest -1000).
  - PE accumulates accum[f, n] += G[e, f].T @ onehot[e, n] in PSUM; the
    gathered tile must be the STATIONARY operand -- the PE's
    moving-operand path crashes when streaming a dma_gather-written tile.
    Rows of G beyond the trimmed gather are stale SBUF data multiplied by
    a zero one-hot column; buffers are memset once so they are never NaN.
  - epilogue per block: ACT copies PSUM->SBUF, one matmul with W.T
    applies the Linear directly on the [f, n] accumulator, ACT scales by
    1/count, DVE adds the bias, DMA writes 128 rows out.
"""

import os
import sys
import types

sys.path.insert(0, "/opt/trn_rl_repo")

import numpy as np

import concourse.bass as bass
import concourse.mybir as mybir
import concourse.tile as tile
from concourse.vector_clock import ScopedClock, VectorClock
from concourse.bass_utils import run_bass_kernel_spmd

# ----------------------------------------------------------------------------
# Environment shims
# ----------------------------------------------------------------------------

def _install_ntff_shim():
    """Register the axon NTFF profile hook if the image's antenv lacks it."""
    try:
        import antenv
    except ImportError:
        return
    if hasattr(antenv, "axon_hooks"):
        return
    hooks_mod = types.ModuleType("antenv.axon_hooks")
    _store = [None]
    hooks_mod.set_axon_ntff_profile_hook = lambda h: _store.__setitem__(0, h)
    hooks_mod.get_axon_ntff_profile_hook = lambda: _store[0]
    sys.modules["antenv.axon_hooks"] = hooks_mod
    antenv.axon_hooks = hooks_mod
    try:
        from trn_agent_boot.trn_boot import _ntff_profile_via_ctypes

        hook = _ntff_profile_via_ctypes("/opt/axon/libaxon_pjrt.so")
        if hook is not None:
            hooks_mod.set_axon_ntff_profile_hook(hook)
    except Exception:
        pass


_install_ntff_shim()


class PatchedTileContext(tile.TileContext):
    """Spread the tail-drain's sem waits over a chain of SP NOPs.

    The walrus build in this container caps sync-waits per instruction
    (setupSyncWait: "Too many sync wait commands"), while stock Tile
    attaches every outstanding proc's wait to one Drain. One NOP per
    proc keeps every instruction at a single wait.
    """

    def _drain_and_barrier(self, tick_clock, wait_clock):
        gc = tick_clock.global_clock
        for p, t in enumerate(gc):
            if t <= 0:
                continue
            nop = self.nc.sync.nop()
            part = VectorClock()
            part.require_at_least(p, t)
            wait_clock.add_sem_waits(nop.ins, ScopedClock({None: part}))
        self.nc.sync.drain()
        self.nc.all_engine_barrier()
        assert self.sems is not None
        popped = self.nc._tile_sem_poison_stack.pop()
        assert popped is self._sem_poison
        self.nc.clear_and_free_semaphores(list(self.sems.allocated().values()))
        self.nc.all_engine_barrier()


# ----------------------------------------------------------------------------
# Problem constants (hardcoded per the task contract)
# ----------------------------------------------------------------------------

N_NODES = 100000
N_CLIQUES = 50000
D = 128
N_CORES = 8
NPC = N_NODES // N_CORES        # 12500 nodes per core
BLK = 128                       # destination nodes per block
NBLK = -(-NPC // BLK)           # 98 blocks per core (last partial: 84)
NPAD = NBLK * BLK               # 12544 padded output rows per core
SPLIT = 32768                   # int16-index limit for dma_gather
PAD_DEST = -1000.0              # one-hot miss value for padding slots

# f32 tables cost nothing on descgen but double DMA bytes and PE time;
# bf16 keeps rel-err ~4e-3 (gate 2e-2). Default bf16, f32 via env.
USE_BF16 = os.environ.get("KERNEL_BF16", "1") == "1"

# SWDGE queues: each dma_gather runs on GpSimd core pair (2q, 2q+1); with
# NQ>1 consecutive gathers go to different pairs and overlap on HW.
NQ = int(os.environ.get("KERNEL_NQ", "4"))
GBUFS = int(os.environ.get("KERNEL_GBUFS", "6"))
# 1: pad to the 128-chunk boundary with index 0, -1 beyond (ucode trims the
# trailing negatives). 0: fill all padding with index 0 (no trim).
TRIM = int(os.environ.get("KERNEL_TRIM", "1"))
# single_packet=True coalesces each ring's gather stream into one DMA packet
# (first/concatenate/last framing), cutting per-packet completion overhead.
SP = os.environ.get("KERNEL_SP", "0") == "1"
# 1: repack nodes into blocks so per-block A/B edge counts land near
# multiples of 128 -- descriptor generation and DMA packets scale with
# ceil(count/128) chunks, so aligned blocks waste nothing.
PACK = int(os.environ.get("KERNEL_PACK", "0"))

_F32 = mybir.dt.float32
_DT = mybir.dt.bfloat16 if USE_BF16 else _F32

if USE_BF16:
    import ml_dtypes

    _NP_DT = np.dtype(ml_dtypes.bfloat16)
else:
    _NP_DT = np.dtype(np.float32)


# ----------------------------------------------------------------------------
# Host-side preparation
# ----------------------------------------------------------------------------

def _pack_slots(a_cnt, b_cnt):
    """Assign each node a slot in the 8x98x128 block grid so that per-block
    A/B edge counts land near multiples of 128 (descgen and DMA cost scale
    with ceil(count/128) 128-row chunks). Greedy best-fit, descending."""
    NBINS = N_CORES * NBLK
    atot, btot = int(a_cnt.sum()), int(b_cnt.sum())
    cha, chb = -(-atot // 128), -(-btot // 128)

    def targets(ch):
        base, extra = divmod(ch, NBINS)
        t = np.full(NBINS, base, dtype=np.int64)
        # spread the +1-chunk bins across cores (bin index is core-major)
        idx = (np.arange(extra) * NBINS // max(extra, 1)) % NBINS
        t[idx] += 1
        return t * 128

    tgtA = targets(cha)
    tgtB = targets(chb)
    remA = tgtA.copy()
    remB = tgtB.copy()
    slots = np.full(NBINS, BLK, dtype=np.int64)

    order = np.argsort(-(a_cnt.astype(np.int64) * 128 + b_cnt))
    slot_map = np.empty(N_NODES, dtype=np.int64)
    fill = np.zeros(NBINS, dtype=np.int64)
    for n in order:
        a, b = a_cnt[n], b_cnt[n]
        score = remA + remB
        feas = (slots > 0) & (remA >= a) & (remB >= b)
        if feas.any():
            i = int(np.argmax(np.where(feas, score, -1)))
        else:
            i = int(np.argmax(np.where(slots > 0, score, -(10**9))))
        slot_map[n] = i * BLK + fill[i]
        fill[i] += 1
        slots[i] -= 1
        remA[i] -= a
        remB[i] -= b
    return slot_map


def _prepare(x_clique, node2clique_index):
    """Sort/bucket/pad the edge list. Returns per-core input dicts plus the
    (data-dependent) tile counts T_A, T_B and the node->slot map."""
    node = np.asarray(node2clique_index[0]).astype(np.int64)
    clique = np.asarray(node2clique_index[1]).astype(np.int64)

    counts = np.bincount(node, minlength=N_NODES).astype(np.float64)
    inv_cnt = (1.0 / np.maximum(counts, 1.0)).astype(np.float32)

    is_a_all = clique < SPLIT
    if PACK:
        a_cnt = np.bincount(node[is_a_all], minlength=N_NODES)
        b_cnt = np.bincount(node[~is_a_all], minlength=N_NODES)
        slot_map = _pack_slots(a_cnt, b_cnt)
    else:
        nid = np.arange(N_NODES)
        slot_map = (nid // NPC) * NPAD + (nid % NPC)

    edge_slot = slot_map[node]
    edge_core = edge_slot // NPAD

    # First pass: per-(core, block) A/B counts to fix the global T_A, T_B.
    per_core = []
    maxA = 0
    maxB = 0
    for c in range(N_CORES):
        sel = np.flatnonzero(edge_core == c)
        es = edge_slot[sel] - c * NPAD
        cq = clique[sel]
        blk = es // BLK
        win = es % BLK
        is_a = cq < SPLIT
        # partition by block, A before B within each block; within each
        # (block, stream) run, ascending clique id gives the gather's DMA
        # reads HBM page locality.
        key = blk * 2 + (~is_a)
        sub = np.lexsort((cq, key))
        blk, win, cq, is_a = blk[sub], win[sub], cq[sub], is_a[sub]
        cntA = np.bincount(blk[is_a], minlength=NBLK)
        cntB = np.bincount(blk[~is_a], minlength=NBLK)
        maxA = max(maxA, int(cntA.max()))
        maxB = max(maxB, int(cntB.max()))
        per_core.append((blk, win, cq, is_a, cntA, cntB))

    T_A = -(-maxA // 128)
    T_B = max(-(-maxB // 128), 1)
    T = T_A + T_B
    LA = T_A * 128
    LB = T_B * 128
    np_dt = _NP_DT

    node_at_slot = np.full(N_CORES * NPAD, -1, dtype=np.int64)
    node_at_slot[slot_map] = np.arange(N_NODES)

    in_maps = []
    for c in range(N_CORES):
        blk, win, cq, is_a, cntA, cntB = per_core[c]

        # -1 padding: the dma_gather ucode trims trailing negative indices,
        # so per-block padding costs no descriptor generation. Real indices
        # are padded with 0 up to the next 128-chunk boundary first, so the
        # trimmed count is always a multiple of 128 and the descriptor
        # generator never sees a partially-valid lane group (untested ucode
        # path on this build).
        idxA = np.full((NBLK, LA), -1, dtype=np.int16)
        idxB = np.full((NBLK, LB), -1, dtype=np.int16)
        dest = np.full((NBLK, T * 128), PAD_DEST, dtype=np.float32)

        offA = np.concatenate([[0], np.cumsum(cntA)])
        offB = np.concatenate([[0], np.cumsum(cntB)])

        a_idx = np.flatnonzero(is_a)
        b_idx = np.flatnonzero(~is_a)
        cqA, winA, blkA = cq[a_idx], win[a_idx], blk[a_idx]
        cqB, winB, blkB = cq[b_idx] - SPLIT, win[b_idx], blk[b_idx]

        posA = np.arange(len(a_idx)) - offA[blkA]
        posB = np.arange(len(b_idx)) - offB[blkB]
        idxA[blkA, posA] = cqA.astype(np.int16)
        idxB[blkB, posB] = cqB.astype(np.int16)
        dest[blkA, posA] = winA
        dest[blkB, posB + LA] = winB

        # pad each block's real indices with 0 (a real gather of row 0;
        # dest stays PAD -> zero one-hot) up to the next 128 multiple;
        # -1 beyond that is trimmed. Also covers empty blocks (cnt=0 ->
        # one full chunk of index 0).
        if TRIM:
            up_a = np.minimum(-(-np.maximum(cntA, 1) // 128) * 128, LA)
            up_b = np.minimum(-(-np.maximum(cntB, 1) // 128) * 128, LB)
        else:
            up_a = np.full(NBLK, LA)
            up_b = np.full(NBLK, LB)
        colA = np.arange(LA)[None, :]
        colB = np.arange(LB)[None, :]
        fillA = (idxA == -1) & (colA < up_a[:, None])
        fillB = (idxB == -1) & (colB < up_b[:, None])
        idxA[fillA] = 0
        idxB[fillB] = 0

        # runtime valid-index counts, one per gather: the decode stage
        # reserves ring slots from num_idxs_reg while the Q7 ucode counts
        # the trimmed indices -- the two MUST match or the ring desyncs
        # and the device hangs.
        cnts = np.empty((1, 2 * NBLK), dtype=np.int32)
        cnts[0, 0::2] = up_a
        cnts[0, 1::2] = up_b

        # wrap indices for dma_gather: seq j -> [j % 16, j // 16], one block
        # per gather call. dma_gather reads a [128, n/16] idx AP: the
        # [16, n/16] wrap is replicated across all 8 GpSimd cores'
        # partition groups.
        def _wrap(idx, L):
            w = idx.reshape(NBLK, -1, 16)
            w = np.ascontiguousarray(np.transpose(w, (2, 0, 1))).reshape(16, -1)
            return np.tile(w, (8, 1))

        wA = _wrap(idxA, LA)
        wB = _wrap(idxB, LB)

        # dest layout for the batched is_equal: [128, NBLK * T]
        dest_t = np.ascontiguousarray(
            dest.reshape(NBLK * T, 128).T
        ).astype(np_dt)

        ns_c = node_at_slot[c * NPAD : (c + 1) * NPAD]
        iv = np.where(ns_c >= 0, inv_cnt[np.maximum(ns_c, 0)], 0.0).astype(
            np.float32
        )
        inv_t = np.ascontiguousarray(iv.reshape(NBLK, BLK).T)

        in_maps.append(
            {
                "idxA": wA,
                "idxB": wB,
                "dest": dest_t,
                "invc": inv_t,
                "cnts": cnts,
            }
        )

    shared = {
        "xcA": np.ascontiguousarray(np.asarray(x_clique)[:SPLIT]).astype(np_dt),
        "xcB": np.ascontiguousarray(np.asarray(x_clique)[SPLIT:]).astype(np_dt),
        "iota": np.tile(np.arange(128, dtype=np.float32), (128, 1)).astype(np_dt),
    }
    return in_maps, shared, T_A, T_B, slot_map


# ----------------------------------------------------------------------------
# Kernel builder
# ----------------------------------------------------------------------------

def _build(T_A, T_B):
    T = T_A + T_B
    LA, LB = T_A * 128, T_B * 128
    CB = N_CLIQUES - SPLIT

    from concourse.bacc import Bacc

    nc = Bacc(None, num_swdge_queues=NQ)
    xcA = nc.declare_dram_parameter("xcA", [SPLIT, D], _DT, isOutput=False)
    xcB = nc.declare_dram_parameter("xcB", [CB, D], _DT, isOutput=False)
    idxA = nc.declare_dram_parameter(
        "idxA", [128, NBLK * LA // 16], mybir.dt.int16, isOutput=False
    )
    idxB = nc.declare_dram_parameter(
        "idxB", [128, NBLK * LB // 16], mybir.dt.int16, isOutput=False
    )
    dest = nc.declare_dram_parameter("dest", [128, NBLK * T], _DT, isOutput=False)
    invc = nc.declare_dram_parameter("invc", [128, NBLK], _F32, isOutput=False)
    cnts = nc.declare_dram_parameter(
        "cnts", [1, 2 * NBLK], mybir.dt.int32, isOutput=False
    )
    iota = nc.declare_dram_parameter("iota", [128, 128], _DT, isOutput=False)
    wt = nc.declare_dram_parameter("wt", [128, 128], _DT, isOutput=False)
    bb = nc.declare_dram_parameter("bb", [128, 128], _F32, isOutput=False)
    out = nc.declare_dram_parameter("out", [NPAD, D], _F32, isOutput=True)

    from contextlib import ExitStack

    with PatchedTileContext(nc) as tc, ExitStack() as ctx:
        const = ctx.enter_context(tc.tile_pool(name="const", bufs=1))
        sb = ctx.enter_context(tc.tile_pool(name="sb", bufs=3))
        gpool = ctx.enter_context(tc.tile_pool(name="g", bufs=GBUFS))
        ps = ctx.enter_context(tc.tile_pool(name="ps", bufs=2, space="PSUM"))

        idxA_t = const.tile([128, NBLK * LA // 16], mybir.dt.int16)
        nc.sync.dma_start(idxA_t[:], idxA[:])
        idxB_t = const.tile([128, NBLK * LB // 16], mybir.dt.int16)
        nc.sync.dma_start(idxB_t[:], idxB[:])

        def idxA_slice(b):
            return idxA_t[:, b * (LA // 16) : (b + 1) * (LA // 16)]

        def idxB_slice(b):
            return idxB_t[:, b * (LB // 16) : (b + 1) * (LB // 16)]
        dest_t = const.tile([128, NBLK * T], _DT)
        nc.sync.dma_start(dest_t[:], dest[:])
        invc_t = const.tile([128, NBLK], _F32)
        nc.sync.dma_start(invc_t[:], invc[:])
        cnts_t = const.tile([1, 2 * NBLK], mybir.dt.int32)
        nc.sync.dma_start(cnts_t[:], cnts[:])
        iota_t = const.tile([128, 128], _DT)
        nc.sync.dma_start(iota_t[:], iota[:])
        wt_t = const.tile([128, 128], _DT)
        nc.sync.dma_start(wt_t[:], wt[:])
        bb_t = const.tile([128, 128], _F32)
        nc.sync.dma_start(bb_t[:], bb[:])

        for b in range(NBLK):
            gA = gpool.tile([128, T_A, 128], _DT, tag="gA")
            gB = gpool.tile([128, T_B, 128], _DT, tag="gB")
            if b < GBUFS:
                # rows past the runtime-trimmed gather stay stale in SBUF;
                # zero each physical buffer once so they are never NaN
                # (stale values only ever meet zero one-hot columns).
                nc.vector.memset(gA[:], 0.0)
                nc.vector.memset(gB[:], 0.0)
            if TRIM:
                cA = nc.gpsimd.value_load(cnts_t[0:1, 2 * b : 2 * b + 1])
                cB = nc.gpsimd.value_load(cnts_t[0:1, 2 * b + 1 : 2 * b + 2])
            else:
                cA, cB = LA, LB
            nc.gpsimd.dma_gather(
                gA[:],
                xcA[:],
                idxA_slice(b),
                LA,
                cA,
                D,
                single_packet=SP,
                queue_num=(3 * b) % NQ if NQ > 1 else 0,
            )
            nc.gpsimd.dma_gather(
                gB[:],
                xcB[:],
                idxB_slice(b),
                LB,
                cB,
                D,
                single_packet=SP,
                queue_num=(3 * b + 1) % NQ if NQ > 1 else 0,
            )
            onehot = sb.tile([128, T, 128], _DT, tag="oh")
            nc.vector.tensor_tensor(
                out=onehot[:],
                in0=dest_t[:, b * T : (b + 1) * T, None].to_broadcast(
                    [128, T, 128]
                ),
                in1=iota_t[:, None, :].to_broadcast([128, T, 128]),
                op=mybir.AluOpType.is_equal,
            )
            # accum[f, n] += G[e, f].T @ onehot[e, n] -- the gathered tile
            # must be the STATIONARY operand (LDWEIGHTS path); the moving
            # path crashes the PE when reading a dma_gather-written tile.
            accum = ps.tile([128, 128], _F32, tag="acc")
            for t in range(T_A):
                nc.tensor.matmul(
                    out=accum[:],
                    lhsT=gA[:, t, :],
                    rhs=onehot[:, t, :],
                    start=(t == 0),
                    stop=False,
                )
            for t in range(T_B):
                nc.tensor.matmul(
                    out=accum[:],
                    lhsT=gB[:, t, :],
                    rhs=onehot[:, T_A + t, :],
                    start=False,
                    stop=(t == T_B - 1),
                )
            # accum is summed.T -- exactly the lhsT the Linear wants.
            acc_sb = sb.tile([128, 128], _DT, tag="accsb")
            nc.scalar.activation(
                acc_sb[:], accum[:], mybir.ActivationFunctionType.Copy
            )
            lin = ps.tile([128, 128], _F32, tag="lin")
            nc.tensor.matmul(
                out=lin[:], lhsT=acc_sb[:], rhs=wt_t[:], start=True, stop=True
            )
            # out[n, o] = lin[n, o] / count[n] + b[o]
            sc = sb.tile([128, 128], _F32, tag="sc")
            nc.scalar.activation(
                sc[:],
                lin[:],
                mybir.ActivationFunctionType.Copy,
                scale=invc_t[:, b : b + 1],
            )
            outs = sb.tile([128, 128], _F32, tag="outs")
            nc.vector.tensor_tensor(
                out=outs[:], in0=sc[:], in1=bb_t[:], op=mybir.AluOpType.add
            )
            nc.sync.dma_start(out[b * 128 : (b + 1) * 128, :], outs[:])

    nc.finalize()
    return nc


_BUILD_CACHE = {}


def kernel(x, x_clique, node2clique_index, W, b, _trace=False, _tmpdir=None):
    in_maps, shared, T_A, T_B, slot_map = _prepare(x_clique, node2clique_index)

    shared["wt"] = np.ascontiguousarray(np.asarray(W, dtype=np.float32).T).astype(
        _NP_DT
    )
    shared["bb"] = np.tile(
        np.asarray(b, dtype=np.float32)[None, :], (128, 1)
    ).astype(np.float32)

    key = (T_A, T_B, USE_BF16, NQ, GBUFS, TRIM, SP, PACK)
    if key not in _BUILD_CACHE:
        _BUILD_CACHE[key] = _build(T_A, T_B)
    nc = _BUILD_CACHE[key]

    full_maps = [dict(m, **shared) for m in in_maps]
    kwargs = {}
    if _trace:
        kwargs = dict(trace=True, tmpdir=_tmpdir)
    res = run_bass_kernel_spmd(nc, full_maps, core_ids=list(range(N_CORES)), **kwargs)

    out_all = np.concatenate(
        [res.results[c]["out"] for c in range(N_CORES)], axis=0
    )
    out = out_all[slot_map].astype(np.float32)
    if _trace:
        return out, res
    return out


# revision 39
# speedup vs baseline: 1.1883x; 1.0164x over previous
"""Trainium2 Bass kernel for Clique2NodeConvBasic (GNN message passing).

Computes, for the fixed problem size N=100000 nodes, C=50000 cliques,
E=1600000 edges, D=128:

    gathered = x_clique[clique_idx]            # [E, 128]
    summed   = segment_sum(gathered, node_idx) # [N, 128]
    mean     = summed / max(count, 1)
    out      = mean @ W.T + b                  # [N, 128]

Sharding: edges are partitioned by destination-node range across the 8
NeuronCores (12500 nodes per core); x_clique and the 128x128 Linear are
replicated. Segment-sum applies locally, no cross-device reduction.

Per-core device algorithm (v2):
  - host sorts edges by destination and buckets them into 98 blocks of
    128 destination nodes; each block's edge list is split by clique id
    at 32768 (dma_gather indices are int16) into an A and a B stream,
    each padded to a fixed tile count (T_A / T_B) with NEGATIVE indices.
    The dma_gather ucode trims trailing negative indices at runtime, so
    the padding costs no descriptor-generation time (the real indices of
    each stream come first and are all >= 0).
  - one dma_gather per (block, stream), 196 per core. dma_gather runs on
    the GpSimd core pair (2q, 2q+1) selected by queue_num; with
    num_swdge_queues=4 and round-robin queue assignment up to 4 gathers
    overlap on disjoint core pairs (measured ~2x+ on HW; descriptor
    generation at ~7.8 ns/row + 535 ns/instruction is the baseline
    bottleneck at 97% GpSimd occupancy).
  - tables are bf16: halves gather payload and doubles PE throughput.
  - a one-hot matrix (edge -> node-within-block) is built with a batched
    DVE is_equal against an iota tile; padding slots miss (dest -1000).
  - PE accumulates accum[f, n] += G[e, f].T @ onehot[e, n] in PSUM; the
    gathered tile must be the STATIONARY operand -- the PE's
    moving-operand path crashes when streaming a dma_gather-written tile.
    Rows of G beyond the trimmed gather are stale SBUF data multiplied by
    a zero one-hot column; buffers are memset once so they are never NaN.
  - epilogue per block: ACT copies PSUM->SBUF, one matmul with W.T
    applies the Linear directly on the [f, n] accumulator, ACT scales by
    1/count, DVE adds the bias, DMA writes 128 rows out.
"""

import os
import sys
import types

sys.path.insert(0, "/opt/trn_rl_repo")

import numpy as np

import concourse.bass as bass
import concourse.mybir as mybir
import concourse.tile as tile
from concourse.vector_clock import ScopedClock, VectorClock
from concourse.bass_utils import run_bass_kernel_spmd

# ----------------------------------------------------------------------------
# Environment shims
# ----------------------------------------------------------------------------

def _install_ntff_shim():
    """Register the axon NTFF profile hook if the image's antenv lacks it."""
    try:
        import antenv
    except ImportError:
        return
    if hasattr(antenv, "axon_hooks"):
        return
    hooks_mod = types.ModuleType("antenv.axon_hooks")
    _store = [None]
    hooks_mod.set_axon_ntff_profile_hook = lambda h: _store.__setitem__(0, h)
    hooks_mod.get_axon_ntff_profile_hook = lambda: _store[0]
    sys.modules["antenv.axon_hooks"] = hooks_mod
    antenv.axon_hooks = hooks_mod
    try:
        from trn_agent_boot.trn_boot import _ntff_profile_via_ctypes

        hook = _ntff_profile_via_ctypes("/opt/axon/libaxon_pjrt.so")
        if hook is not None:
            hooks_mod.set_axon_ntff_profile_hook(hook)
    except Exception:
        pass


_install_ntff_shim()


class PatchedTileContext(tile.TileContext):
    """Spread the tail-drain's sem waits over a chain of SP NOPs.

    The walrus build in this container caps sync-waits per instruction
    (setupSyncWait: "Too many sync wait commands"), while stock Tile
    attaches every outstanding proc's wait to one Drain. One NOP per
    proc keeps every instruction at a single wait.
    """

    def _drain_and_barrier(self, tick_clock, wait_clock):
        gc = tick_clock.global_clock
        for p, t in enumerate(gc):
            if t <= 0:
                continue
            nop = self.nc.sync.nop()
            part = VectorClock()
            part.require_at_least(p, t)
            wait_clock.add_sem_waits(nop.ins, ScopedClock({None: part}))
        self.nc.sync.drain()
        self.nc.all_engine_barrier()
        assert self.sems is not None
        popped = self.nc._tile_sem_poison_stack.pop()
        assert popped is self._sem_poison
        self.nc.clear_and_free_semaphores(list(self.sems.allocated().values()))
        self.nc.all_engine_barrier()


# ----------------------------------------------------------------------------
# Problem constants (hardcoded per the task contract)
# ----------------------------------------------------------------------------

N_NODES = 100000
N_CLIQUES = 50000
D = 128
N_CORES = 8
NPC = N_NODES // N_CORES        # 12500 nodes per core
BLK = 128                       # destination nodes per block
NBLK = -(-NPC // BLK)           # 98 blocks per core (last partial: 84)
NPAD = NBLK * BLK               # 12544 padded output rows per core
SPLIT = 32768                   # int16-index limit for dma_gather
PAD_DEST = -1000.0              # one-hot miss value for padding slots

# f32 tables cost nothing on descgen but double DMA bytes and PE time;
# bf16 keeps rel-err ~4e-3 (gate 2e-2). Default bf16, f32 via env.
USE_BF16 = os.environ.get("KERNEL_BF16", "1") == "1"

# SWDGE queues: each dma_gather runs on GpSimd core pair (2q, 2q+1); with
# NQ>1 consecutive gathers go to different pairs and overlap on HW.
NQ = int(os.environ.get("KERNEL_NQ", "4"))
GBUFS = int(os.environ.get("KERNEL_GBUFS", "6"))
# 1: pad to the 128-chunk boundary with index 0, -1 beyond (ucode trims the
# trailing negatives). 0: fill all padding with index 0 (no trim).
TRIM = int(os.environ.get("KERNEL_TRIM", "1"))
# single_packet=True coalesces each ring's gather stream into one DMA packet
# (first/concatenate/last framing), cutting per-packet completion overhead.
SP = os.environ.get("KERNEL_SP", "0") == "1"
# 1: repack nodes into blocks so per-block A/B edge counts land near
# multiples of 128 -- descriptor generation and DMA packets scale with
# ceil(count/128) chunks, so aligned blocks waste nothing.
PACK = int(os.environ.get("KERNEL_PACK", "0"))

_F32 = mybir.dt.float32
_DT = mybir.dt.bfloat16 if USE_BF16 else _F32

if USE_BF16:
    import ml_dtypes

    _NP_DT = np.dtype(ml_dtypes.bfloat16)
else:
    _NP_DT = np.dtype(np.float32)


# ----------------------------------------------------------------------------
# Host-side preparation
# ----------------------------------------------------------------------------

def _pack_slots(a_cnt, b_cnt):
    """Assign each node a slot in the 8x98x128 block grid so that per-block
    A/B edge counts land just under multiples of 128: descgen and DMA cost
    scale with ceil(count/128) chunks, so a block summing anywhere in
    (target-128, target] costs exactly target/128 chunks (overshoot by one
    edge costs a whole extra chunk, undershoot is free). Also caps the
    maxima so the static tile counts T_A/T_B shrink -- the gather ucode's
    index-load phase scales with the static padded length."""
    NBINS = N_CORES * NBLK
    atot, btot = int(a_cnt.sum()), int(b_cnt.sum())
    cha, chb = -(-atot // 128), -(-btot // 128)

    def targets(ch):
        base, extra = divmod(ch, NBINS)
        t = np.full(NBINS, base, dtype=np.int64)
        idx = (np.arange(extra, dtype=np.int64) * NBINS) // max(extra, 1)
        t[np.minimum(idx, NBINS - 1)] += 1
        return t * 128

    tgtA = targets(cha)
    # pair high A targets with low B targets so total bin load stays even
    tgtB = targets(chb)[::-1].copy()

    # per-a-value buckets, each sorted by b ascending; pick from either end
    from collections import deque

    amax = int(a_cnt.max())
    order = np.lexsort((b_cnt, a_cnt))
    a_sorted = a_cnt[order]
    buckets = {}
    for v in range(amax + 1):
        lo = np.searchsorted(a_sorted, v, side="left")
        hi = np.searchsorted(a_sorted, v, side="right")
        if hi > lo:
            buckets[v] = deque(order[lo:hi].tolist())

    slot_map = np.empty(N_NODES, dtype=np.int64)
    n_left = N_NODES
    for i in range(NBINS):
        SA = SB = 0
        want = min(BLK, n_left)
        filled = 0
        while filled < want and n_left > 0:
            k = want - filled
            # aim at the middle of the free window (target-64)
            padA = (tgtA[i] - 64 - SA) / k
            padB = (tgtB[i] - 64 - SB) / k
            capA = tgtA[i] - SA
            best = None
            bestd = None
            for v in buckets:
                d = abs(v - padA) + (10**6 if v > capA else 0)
                if bestd is None or d < bestd:
                    bestd = d
                    best = v
            dq = buckets[best]
            # take from the b-end that tracks the B pace
            n = dq.pop() if b_cnt[dq[-1]] <= padB else dq.popleft()
            if not dq:
                del buckets[best]
            slot_map[n] = i * BLK + filled
            SA += a_cnt[n]
            SB += b_cnt[n]
            filled += 1
            n_left -= 1
    return slot_map


def _prepare(x_clique, node2clique_index):
    """Sort/bucket/pad the edge list. Returns per-core input dicts plus the
    (data-dependent) tile counts T_A, T_B and the node->slot map."""
    node = np.asarray(node2clique_index[0]).astype(np.int64)
    clique = np.asarray(node2clique_index[1]).astype(np.int64)

    counts = np.bincount(node, minlength=N_NODES).astype(np.float64)
    inv_cnt = (1.0 / np.maximum(counts, 1.0)).astype(np.float32)

    is_a_all = clique < SPLIT
    if PACK:
        a_cnt = np.bincount(node[is_a_all], minlength=N_NODES)
        b_cnt = np.bincount(node[~is_a_all], minlength=N_NODES)
        slot_map = _pack_slots(a_cnt, b_cnt)
    else:
        nid = np.arange(N_NODES)
        slot_map = (nid // NPC) * NPAD + (nid % NPC)

    edge_slot = slot_map[node]
    edge_core = edge_slot // NPAD

    # First pass: per-(core, block) A/B counts to fix the global T_A, T_B.
    per_core = []
    maxA = 0
    maxB = 0
    for c in range(N_CORES):
        sel = np.flatnonzero(edge_core == c)
        es = edge_slot[sel] - c * NPAD
        cq = clique[sel]
        blk = es // BLK
        win = es % BLK
        is_a = cq < SPLIT
        # partition by block, A before B within each block; within each
        # (block, stream) run, ascending clique id gives the gather's DMA
        # reads HBM page locality.
        key = blk * 2 + (~is_a)
        sub = np.lexsort((cq, key))
        blk, win, cq, is_a = blk[sub], win[sub], cq[sub], is_a[sub]
        cntA = np.bincount(blk[is_a], minlength=NBLK)
        cntB = np.bincount(blk[~is_a], minlength=NBLK)
        maxA = max(maxA, int(cntA.max()))
        maxB = max(maxB, int(cntB.max()))
        per_core.append((blk, win, cq, is_a, cntA, cntB))

    T_A = -(-maxA // 128)
    T_B = max(-(-maxB // 128), 1)
    T = T_A + T_B
    LA = T_A * 128
    LB = T_B * 128
    np_dt = _NP_DT

    node_at_slot = np.full(N_CORES * NPAD, -1, dtype=np.int64)
    node_at_slot[slot_map] = np.arange(N_NODES)

    in_maps = []
    for c in range(N_CORES):
        blk, win, cq, is_a, cntA, cntB = per_core[c]

        # -1 padding: the dma_gather ucode trims trailing negative indices,
        # so per-block padding costs no descriptor generation. Real indices
        # are padded with 0 up to the next 128-chunk boundary first, so the
        # trimmed count is always a multiple of 128 and the descriptor
        # generator never sees a partially-valid lane group (untested ucode
        # path on this build).
        idxA = np.full((NBLK, LA), -1, dtype=np.int16)
        idxB = np.full((NBLK, LB), -1, dtype=np.int16)
        dest = np.full((NBLK, T * 128), PAD_DEST, dtype=np.float32)

        offA = np.concatenate([[0], np.cumsum(cntA)])
        offB = np.concatenate([[0], np.cumsum(cntB)])

        a_idx = np.flatnonzero(is_a)
        b_idx = np.flatnonzero(~is_a)
        cqA, winA, blkA = cq[a_idx], win[a_idx], blk[a_idx]
        cqB, winB, blkB = cq[b_idx] - SPLIT, win[b_idx], blk[b_idx]

        posA = np.arange(len(a_idx)) - offA[blkA]
        posB = np.arange(len(b_idx)) - offB[blkB]
        idxA[blkA, posA] = cqA.astype(np.int16)
        idxB[blkB, posB] = cqB.astype(np.int16)
        dest[blkA, posA] = winA
        dest[blkB, posB + LA] = winB

        # pad each block's real indices with 0 (a real gather of row 0;
        # dest stays PAD -> zero one-hot) up to the next 128 multiple;
        # -1 beyond that is trimmed. Also covers empty blocks (cnt=0 ->
        # one full chunk of index 0).
        if TRIM:
            up_a = np.minimum(-(-np.maximum(cntA, 1) // 128) * 128, LA)
            up_b = np.minimum(-(-np.maximum(cntB, 1) // 128) * 128, LB)
        else:
            up_a = np.full(NBLK, LA)
            up_b = np.full(NBLK, LB)
        colA = np.arange(LA)[None, :]
        colB = np.arange(LB)[None, :]
        fillA = (idxA == -1) & (colA < up_a[:, None])
        fillB = (idxB == -1) & (colB < up_b[:, None])
        idxA[fillA] = 0
        idxB[fillB] = 0

        # runtime valid-index counts, one per gather: the decode stage
        # reserves ring slots from num_idxs_reg while the Q7 ucode counts
        # the trimmed indices -- the two MUST match or the ring desyncs
        # and the device hangs.
        cnts = np.empty((1, 2 * NBLK), dtype=np.int32)
        cnts[0, 0::2] = up_a
        cnts[0, 1::2] = up_b

        # wrap indices for dma_gather: seq j -> [j % 16, j // 16], one block
        # per gather call. dma_gather reads a [128, n/16] idx AP: the
        # [16, n/16] wrap is replicated across all 8 GpSimd cores'
        # partition groups.
        def _wrap(idx, L):
            w = idx.reshape(NBLK, -1, 16)
            w = np.ascontiguousarray(np.transpose(w, (2, 0, 1))).reshape(16, -1)
            return np.tile(w, (8, 1))

        wA = _wrap(idxA, LA)
        wB = _wrap(idxB, LB)

        # dest layout for the batched is_equal: [128, NBLK * T]
        dest_t = np.ascontiguousarray(
            dest.reshape(NBLK * T, 128).T
        ).astype(np_dt)

        ns_c = node_at_slot[c * NPAD : (c + 1) * NPAD]
        iv = np.where(ns_c >= 0, inv_cnt[np.maximum(ns_c, 0)], 0.0).astype(
            np.float32
        )
        inv_t = np.ascontiguousarray(iv.reshape(NBLK, BLK).T)

        in_maps.append(
            {
                "idxA": wA,
                "idxB": wB,
                "dest": dest_t,
                "invc": inv_t,
                "cnts": cnts,
            }
        )

    shared = {
        "xcA": np.ascontiguousarray(np.asarray(x_clique)[:SPLIT]).astype(np_dt),
        "xcB": np.ascontiguousarray(np.asarray(x_clique)[SPLIT:]).astype(np_dt),
        "iota": np.tile(np.arange(128, dtype=np.float32), (128, 1)).astype(np_dt),
    }
    return in_maps, shared, T_A, T_B, slot_map


# ----------------------------------------------------------------------------
# Kernel builder
# ----------------------------------------------------------------------------

def _build(T_A, T_B):
    T = T_A + T_B
    LA, LB = T_A * 128, T_B * 128
    CB = N_CLIQUES - SPLIT

    from concourse.bacc import Bacc

    nc = Bacc(None, num_swdge_queues=NQ)
    xcA = nc.declare_dram_parameter("xcA", [SPLIT, D], _DT, isOutput=False)
    xcB = nc.declare_dram_parameter("xcB", [CB, D], _DT, isOutput=False)
    idxA = nc.declare_dram_parameter(
        "idxA", [128, NBLK * LA // 16], mybir.dt.int16, isOutput=False
    )
    idxB = nc.declare_dram_parameter(
        "idxB", [128, NBLK * LB // 16], mybir.dt.int16, isOutput=False
    )
    dest = nc.declare_dram_parameter("dest", [128, NBLK * T], _DT, isOutput=False)
    invc = nc.declare_dram_parameter("invc", [128, NBLK], _F32, isOutput=False)
    cnts = nc.declare_dram_parameter(
        "cnts", [1, 2 * NBLK], mybir.dt.int32, isOutput=False
    )
    iota = nc.declare_dram_parameter("iota", [128, 128], _DT, isOutput=False)
    wt = nc.declare_dram_parameter("wt", [128, 128], _DT, isOutput=False)
    bb = nc.declare_dram_parameter("bb", [128, 128], _F32, isOutput=False)
    out = nc.declare_dram_parameter("out", [NPAD, D], _F32, isOutput=True)

    from contextlib import ExitStack

    with PatchedTileContext(nc) as tc, ExitStack() as ctx:
        const = ctx.enter_context(tc.tile_pool(name="const", bufs=1))
        sb = ctx.enter_context(tc.tile_pool(name="sb", bufs=3))
        gpool = ctx.enter_context(tc.tile_pool(name="g", bufs=GBUFS))
        ps = ctx.enter_context(tc.tile_pool(name="ps", bufs=2, space="PSUM"))

        # two-stage idx load: a small head unblocks the first gathers while
        # the bulk streams in behind it.
        HEAD = 12
        idxA_t = const.tile([128, NBLK * LA // 16], mybir.dt.int16)
        nc.sync.dma_start(
            idxA_t[:, : HEAD * (LA // 16)], idxA[:, : HEAD * (LA // 16)]
        )
        idxB_t = const.tile([128, NBLK * LB // 16], mybir.dt.int16)
        nc.sync.dma_start(
            idxB_t[:, : HEAD * (LB // 16)], idxB[:, : HEAD * (LB // 16)]
        )
        nc.sync.dma_start(
            idxA_t[:, HEAD * (LA // 16) :], idxA[:, HEAD * (LA // 16) :]
        )
        nc.sync.dma_start(
            idxB_t[:, HEAD * (LB // 16) :], idxB[:, HEAD * (LB // 16) :]
        )

        def idxA_slice(b):
            return idxA_t[:, b * (LA // 16) : (b + 1) * (LA // 16)]

        def idxB_slice(b):
            return idxB_t[:, b * (LB // 16) : (b + 1) * (LB // 16)]
        dest_t = const.tile([128, NBLK * T], _DT)
        nc.sync.dma_start(dest_t[:], dest[:])
        invc_t = const.tile([128, NBLK], _F32)
        nc.sync.dma_start(invc_t[:], invc[:])
        cnts_t = const.tile([1, 2 * NBLK], mybir.dt.int32)
        nc.sync.dma_start(cnts_t[:], cnts[:])
        iota_t = const.tile([128, 128], _DT)
        nc.sync.dma_start(iota_t[:], iota[:])
        wt_t = const.tile([128, 128], _DT)
        nc.sync.dma_start(wt_t[:], wt[:])
        bb_t = const.tile([128, 128], _F32)
        nc.sync.dma_start(bb_t[:], bb[:])

        for b in range(NBLK):
            gA = gpool.tile([128, T_A, 128], _DT, tag="gA")
            gB = gpool.tile([128, T_B, 128], _DT, tag="gB")
            if b < GBUFS:
                # rows past the runtime-trimmed gather stay stale in SBUF;
                # zero each physical buffer once so they are never NaN
                # (stale values only ever meet zero one-hot columns).
                nc.vector.memset(gA[:], 0.0)
                nc.vector.memset(gB[:], 0.0)
            if TRIM:
                cA = nc.gpsimd.value_load(cnts_t[0:1, 2 * b : 2 * b + 1])
                cB = nc.gpsimd.value_load(cnts_t[0:1, 2 * b + 1 : 2 * b + 2])
            else:
                cA, cB = LA, LB
            nc.gpsimd.dma_gather(
                gA[:],
                xcA[:],
                idxA_slice(b),
                LA,
                cA,
                D,
                single_packet=SP,
                queue_num=(3 * b) % NQ if NQ > 1 else 0,
            )
            nc.gpsimd.dma_gather(
                gB[:],
                xcB[:],
                idxB_slice(b),
                LB,
                cB,
                D,
                single_packet=SP,
                queue_num=(3 * b + 1) % NQ if NQ > 1 else 0,
            )
            onehot = sb.tile([128, T, 128], _DT, tag="oh")
            nc.vector.tensor_tensor(
                out=onehot[:],
                in0=dest_t[:, b * T : (b + 1) * T, None].to_broadcast(
                    [128, T, 128]
                ),
                in1=iota_t[:, None, :].to_broadcast([128, T, 128]),
                op=mybir.AluOpType.is_equal,
            )
            # accum[f, n] += G[e, f].T @ onehot[e, n] -- the gathered tile
            # must be the STATIONARY operand (LDWEIGHTS path); the moving
            # path crashes the PE when reading a dma_gather-written tile.
            accum = ps.tile([128, 128], _F32, tag="acc")
            for t in range(T_A):
                nc.tensor.matmul(
                    out=accum[:],
                    lhsT=gA[:, t, :],
                    rhs=onehot[:, t, :],
                    start=(t == 0),
                    stop=False,
                )
            for t in range(T_B):
                nc.tensor.matmul(
                    out=accum[:],
                    lhsT=gB[:, t, :],
                    rhs=onehot[:, T_A + t, :],
                    start=False,
                    stop=(t == T_B - 1),
                )
            # accum is summed.T -- exactly the lhsT the Linear wants.
            acc_sb = sb.tile([128, 128], _DT, tag="accsb")
            nc.scalar.activation(
                acc_sb[:], accum[:], mybir.ActivationFunctionType.Copy
            )
            lin = ps.tile([128, 128], _F32, tag="lin")
            nc.tensor.matmul(
                out=lin[:], lhsT=acc_sb[:], rhs=wt_t[:], start=True, stop=True
            )
            # out[n, o] = lin[n, o] / count[n] + b[o]
            sc = sb.tile([128, 128], _F32, tag="sc")
            nc.scalar.activation(
                sc[:],
                lin[:],
                mybir.ActivationFunctionType.Copy,
                scale=invc_t[:, b : b + 1],
            )
            outs = sb.tile([128, 128], _F32, tag="outs")
            nc.vector.tensor_tensor(
                out=outs[:], in0=sc[:], in1=bb_t[:], op=mybir.AluOpType.add
            )
            nc.sync.dma_start(out[b * 128 : (b + 1) * 128, :], outs[:])

    nc.finalize()
    return nc


_BUILD_CACHE = {}


def kernel(x, x_clique, node2clique_index, W, b, _trace=False, _tmpdir=None):
    in_maps, shared, T_A, T_B, slot_map = _prepare(x_clique, node2clique_index)

    shared["wt"] = np.ascontiguousarray(np.asarray(W, dtype=np.float32).T).astype(
        _NP_DT
    )
    shared["bb"] = np.tile(
        np.asarray(b, dtype=np.float32)[None, :], (128, 1)
    ).astype(np.float32)

    key = (T_A, T_B, USE_BF16, NQ, GBUFS, TRIM, SP, PACK)
    if key not in _BUILD_CACHE:
        _BUILD_CACHE[key] = _build(T_A, T_B)
    nc = _BUILD_CACHE[key]

    full_maps = [dict(m, **shared) for m in in_maps]
    kwargs = {}
    if _trace:
        kwargs = dict(trace=True, tmpdir=_tmpdir)
    res = run_bass_kernel_spmd(nc, full_maps, core_ids=list(range(N_CORES)), **kwargs)

    out_all = np.concatenate(
        [res.results[c]["out"] for c in range(N_CORES)], axis=0
    )
    out = out_all[slot_map].astype(np.float32)
    if _trace:
        return out, res
    return out
